# revision 11
# baseline (speedup 1.0000x reference)
"""Causal self-attention (B=4, S=2048, D=2048, H=16) on 8 TRN2 NeuronCores.

Sharding: core c -> batch b=c//2, tensor-parallel half t=c%2 (8 heads each).
Each core computes QKV projections for its 8 heads, causal attention, and a
partial out-projection; the host sums the two TP partials per batch.

All heavy matmuls run in float32r (E8M11, 4x faster than fp32 on the PE,
~1.5e-4 relative rounding), accumulating in fp32 PSUM. Inputs are pre-rounded
to the f32r grid on the host (fp32 with the low 12 mantissa bits rounded
away), so no on-chip dtype conversions are needed.
"""
import math
import numpy as np

import concourse.bass as bass
import concourse.bacc as bacc
import concourse.mybir as mybir
import concourse.tile as tile
from concourse.bass_utils import run_bass_kernel_spmd

B, S, D, H, HD = 4, 2048, 2048, 16, 128
HL = 8              # heads per core
ML = HL * HD        # local model dims (1024)
P = 128
NDT = D // P        # 16 contraction tiles
NST = S // P        # 16 seq tiles
NSC = S // 512      # 4 seq chunks
ISQ = 1.0 / math.sqrt(HD)
F32 = mybir.dt.float32
F32R = mybir.dt.float32r
Exp = mybir.ActivationFunctionType.Exp

_BUILT = {}


def _build():
    nc = bacc.Bacc("TRN2", target_bir_lowering=False, debug=False, num_devices=8)
    xT = nc.declare_dram_parameter("xT", [D, S], F32R, isOutput=False)
    wqT = nc.declare_dram_parameter("wqT", [D, ML], F32R, isOutput=False)
    wkT = nc.declare_dram_parameter("wkT", [D, ML], F32R, isOutput=False)
    wvT = nc.declare_dram_parameter("wvT", [D, ML], F32R, isOutput=False)
    woT = nc.declare_dram_parameter("woT", [ML, D], F32R, isOutput=False)
    mask0 = nc.declare_dram_parameter("mask0", [P, 896], F32, isOutput=False)
    out = nc.declare_dram_parameter("out", [S, D], F32, isOutput=True)

    with tile.TileContext(nc) as tc:
        with tc.tile_pool(name="dram", bufs=1, space="DRAM") as dp:
            # per-head / per-quarter scratch so phase-2 loads depend only on
            # the slices actually produced (fine-grained cross-phase overlap)
            sq = [dp.tile([P, S], F32R, tag=f"sq{h}", name=f"sq{h}")
                  for h in range(HL)]
            sk = [dp.tile([P, S], F32R, tag=f"sk{h}", name=f"sk{h}")
                  for h in range(HL)]
            svq = [dp.tile([S, 256], F32R, tag=f"sv{m}", name=f"sv{m}")
                   for m in range(4)]

            xT_r = xT.ap().rearrange("(t p) s -> p t s", p=P)
            wqT_r = wqT.ap().rearrange("(t p) m -> p t m", p=P)
            wkT_r = wkT.ap().rearrange("(t p) m -> p t m", p=P)
            wvT_r = wvT.ap().rearrange("(t p) m -> p t m", p=P)

            # ---------------- Phase 1: QKV projections ----------------
            with tc.tile_pool(name="xr_pool", bufs=1) as xr_pool, \
                 tc.tile_pool(name="p1ev", bufs=3) as p1ev, \
                 tc.tile_pool(name="p1ps", bufs=2, space="PSUM") as p1ps:
                # x^T resident f32r [128, 16 d-tiles, 2048 s], loaded by
                # s-chunk so the first matmul group can start after ~4MiB.
                xr = xr_pool.tile([P, NDT, S], F32R, tag="xr")
                for c in range(NSC):
                    nc.sync.dma_start(
                        out=xr[:, :, c * 512:(c + 1) * 512],
                        in_=xT_r[:, :, c * 512:(c + 1) * 512])

                # v first (phase 2's head-0 needs v quarter 0 early):
                # v[s-tile 128, m-quarter 256]; stationary = x, moving = Wv
                with tc.tile_pool(name="vw", bufs=2) as vw:
                    for mq in range(4):
                        vr = vw.tile([P, NDT, 256], F32R, tag="vr")
                        nc.sync.dma_start(
                            out=vr[:], in_=wvT_r[:, :, mq * 256:(mq + 1) * 256])
                        for st in range(NST):
                            ps = p1ps.tile([P, 256], F32, tag="pv")
                            for d in range(NDT):
                                nc.tensor.matmul(
                                    ps[:], xr[:, d, st * P:(st + 1) * P],
                                    vr[:, d, :],
                                    start=(d == 0), stop=(d == NDT - 1))
                            ev = p1ev.tile([P, 256], F32R, tag="evv")
                            nc.scalar.copy(ev[:], ps[:])
                            nc.sync.dma_start(
                                out=svq[mq][st * P:(st + 1) * P, :], in_=ev[:])

                # q^T / k^T per head: [m-tile 128, s]; stationary = W slice
                with tc.tile_pool(name="qkw", bufs=3) as qkw:
                    for h in range(HL):
                        for wsrc, dst in ((wqT_r, sq[h]), (wkT_r, sk[h])):
                            wr = qkw.tile([P, NDT, P], F32R, tag="wr")
                            nc.sync.dma_start(
                                out=wr[:], in_=wsrc[:, :, h * P:(h + 1) * P])
                            for c in range(NSC):
                                ps = p1ps.tile([P, 512], F32, tag="pp")
                                for d in range(NDT):
                                    nc.tensor.matmul(
                                        ps[:], wr[:, d, :],
                                        xr[:, d, c * 512:(c + 1) * 512],
                                        start=(d == 0), stop=(d == NDT - 1))
                                ev = p1ev.tile([P, 512], F32R, tag="ev")
                                nc.scalar.copy(ev[:], ps[:])
                                nc.sync.dma_start(
                                    out=dst[:, c * 512:(c + 1) * 512],
                                    in_=ev[:])

            # ---------------- Phase 2: attention per head ----------------
            with tc.tile_pool(name="ctx_pool", bufs=1) as ctx_pool, \
                 tc.tile_pool(name="wo_pool", bufs=1) as wo_pool:
                ctx = ctx_pool.tile([P, HL, S], F32R, tag="ctx")
                # prefetch out-projection weights during attention
                woT_r = woT.ap().rearrange("(h p) n -> p h n", p=P)
                wo = wo_pool.tile([P, HL, D], F32R, tag="wo")
                for h in range(HL):
                    nc.sync.dma_start(out=wo[:, h, :], in_=woT_r[:, h, :])

                with tc.tile_pool(name="const", bufs=1) as const, \
                     tc.tile_pool(name="qkv", bufs=2) as qkv, \
                     tc.tile_pool(name="p2w", bufs=2) as p2w, \
                     tc.tile_pool(name="p2ps", bufs=2, space="PSUM") as p2ps, \
                     tc.tile_pool(name="p2ps3", bufs=3, space="PSUM") as p2ps3, \
                     tc.tile_pool(name="p2ps1", bufs=1, space="PSUM") as p2ps1:
                    m0 = const.tile([P, 896], F32, tag="m0")
                    nc.sync.dma_start(out=m0[:], in_=mask0[:])
                    ones_f = const.tile([P, 1], F32, tag="ones_f")
                    nc.vector.memset(ones_f[:], 1.0)
                    ones_col = const.tile([P, 1], F32R, tag="ones_col")
                    nc.vector.tensor_copy(ones_col[:], ones_f[:])
                    onesr_f = const.tile([1, P], F32, tag="onesr_f")
                    nc.vector.memset(onesr_f[:], 1.0)
                    ones_row = const.tile([1, P], F32R, tag="ones_row")
                    nc.vector.tensor_copy(ones_row[:], onesr_f[:])

                    for h in range(HL):
                        q_sb = qkv.tile([P, S], F32R, tag="q_sb")
                        nc.sync.dma_start(out=q_sb[:], in_=sq[h][:])
                        k_sb = qkv.tile([P, S], F32R, tag="k_sb")
                        nc.sync.dma_start(out=k_sb[:], in_=sk[h][:])
                        v_sb = qkv.tile([P, NST, P], F32R, tag="v_sb")
                        nc.sync.dma_start(
                            out=v_sb[:],
                            in_=svq[h // 2][:].rearrange(
                                "(t p) m -> p t m", p=P)[
                                :, :, (h % 2) * P:(h % 2) * P + P])

                        for c in range(NSC):
                            nj = 4 * c + 4
                            pctx = p2ps.tile([P, 512], F32, tag="pctx")
                            pden = p2ps.tile([1, 512], F32, tag="pden")
                            for J in range(nj):
                                diag = J >= 4 * c
                                r = J * P - c * 512 if diag else 0
                                pscr = p2ps3.tile([P, 512], F32, tag="pscr")
                                nc.tensor.matmul(
                                    pscr[:, r:512],
                                    k_sb[:, J * P:(J + 1) * P],
                                    q_sb[:, c * 512 + r:(c + 1) * 512],
                                    start=True, stop=True)
                                pt = p2w.tile([P, 512], F32R, tag="pt",
                                              bufs=4)
                                nc.scalar.activation(pt[:, r:512],
                                                     pscr[:, r:512], Exp,
                                                     scale=ISQ)
                                if diag:
                                    # lower-triangle keep mask on the 128-wide
                                    # diagonal block
                                    nc.vector.tensor_mul(
                                        pt[:, r:r + P], pt[:, r:r + P],
                                        m0[:, 384:512])
                                # suffix-sliced accumulation: columns [0, r)
                                # get no contribution from this J
                                nc.tensor.matmul(
                                    pctx[:, r:512], v_sb[:, J, :],
                                    pt[:, r:512],
                                    start=(J == 0), stop=(J == nj - 1))
                                nc.tensor.matmul(
                                    pden[:, r:512], ones_col[:],
                                    pt[:, r:512],
                                    start=(J == 0), stop=(J == nj - 1))
                            # 1/denominator, broadcast to 128 partitions
                            recip = p2w.tile([1, 512], F32, tag="recip")
                            nc.vector.reciprocal(recip[:], pden[:])
                            recip_r = p2w.tile([1, 512], F32R, tag="recip_r")
                            nc.vector.tensor_copy(recip_r[:], recip[:])
                            pbc = p2ps1.tile([P, 512], F32, tag="pbc")
                            nc.tensor.matmul(pbc[:], ones_row[:], recip_r[:],
                                             start=True, stop=True)
                            rb = p2w.tile([P, 512], F32, tag="rb")
                            nc.vector.tensor_copy(rb[:], pbc[:])
                            nc.vector.tensor_mul(
                                ctx[:, h, c * 512:(c + 1) * 512],
                                pctx[:], rb[:])

                # ---------------- Phase 3: out-projection ----------------
                with tc.tile_pool(name="p3ev", bufs=3) as p3ev, \
                     tc.tile_pool(name="p3ps", bufs=2, space="PSUM") as p3ps:
                    for st in range(NST):
                        for nk in range(NSC):
                            ps = p3ps.tile([P, 512], F32, tag="po")
                            for h in range(HL):
                                nc.tensor.matmul(
                                    ps[:], ctx[:, h, st * P:(st + 1) * P],
                                    wo[:, h, nk * 512:(nk + 1) * 512],
                                    start=(h == 0), stop=(h == HL - 1))
                            ev = p3ev.tile([P, 512], F32, tag="evo")
                            nc.scalar.copy(ev[:], ps[:])
                            nc.sync.dma_start(
                                out=out[st * P:(st + 1) * P,
                                        nk * 512:(nk + 1) * 512], in_=ev[:])

    nc.finalize()
    return nc


def get_nc():
    if "nc" not in _BUILT:
        _BUILT["nc"] = _build()
    return _BUILT["nc"]


def _to_f32r(a):
    """Round fp32 to the float32r (E8M11) grid: RNE at 12 low mantissa bits."""
    u = np.ascontiguousarray(a, dtype=np.float32).view(np.uint32)
    r = (u + 0x7FF + ((u >> 12) & 1)) & np.uint32(0xFFFFF000)
    return r.view(np.float32)


def _make_in_maps(x, Wq, Wk, Wv, Wo):
    jj, tt = np.meshgrid(np.arange(P), np.arange(896), indexing="ij")
    mask0 = (tt >= jj + 384).astype(np.float32)
    in_maps = []
    for c in range(8):
        b, t = c // 2, c % 2
        ms = slice(t * ML, (t + 1) * ML)
        in_maps.append({
            "xT": _to_f32r(x[b].T),
            "wqT": _to_f32r(Wq[ms, :].T),
            "wkT": _to_f32r(Wk[ms, :].T),
            "wvT": _to_f32r(Wv[ms, :].T),
            "woT": _to_f32r(Wo[:, ms].T),
            "mask0": mask0,
        })
    return in_maps


def kernel(x, Wq, Wk, Wv, Wo):
    x = np.asarray(x, dtype=np.float32)
    Wq = np.asarray(Wq, dtype=np.float32)
    Wk = np.asarray(Wk, dtype=np.float32)
    Wv = np.asarray(Wv, dtype=np.float32)
    Wo = np.asarray(Wo, dtype=np.float32)

    nc = get_nc()
    in_maps = _make_in_maps(x, Wq, Wk, Wv, Wo)
    res = run_bass_kernel_spmd(nc, in_maps, list(range(8)))
    outs = [res.results[c]["out"] for c in range(8)]
    full = np.stack([outs[2 * b] + outs[2 * b + 1] for b in range(B)])
    return full.astype(np.float32)


# revision 15
# speedup vs baseline: 1.5323x; 1.5323x over previous
"""Causal self-attention (B=4, S=2048, D=2048, H=16) on 8 TRN2 NeuronCores.

Sharding: core c -> batch b=c//2, tensor-parallel half t=c%2 (8 heads each).
Each core computes QKV projections for its 8 heads, causal attention, and a
partial out-projection; the host sums the two TP partials per batch.

All heavy matmuls run in float32r (E8M11, 4x faster than fp32 on the PE,
~1.5e-4 relative rounding), accumulating in fp32 PSUM. Inputs are pre-rounded
to the f32r grid on the host, so no on-chip dtype conversions are needed.

Projections run in two sequence halves (x^T half resident at a time); since
causal attention on chunk c consumes only projections of chunks <= c, the
attention pass over the first half is emitted (and scheduled) between the two
projection halves. Phases communicate via per-(head, chunk) DRAM scratch;
attention context is spilled to DRAM and re-read by the out-projection, whose
weights prefetch into the space vacated by x^T.
"""
import math
from contextlib import ExitStack

import numpy as np

import concourse.bass as bass
import concourse.bacc as bacc
import concourse.mybir as mybir
import concourse.tile as tile
from concourse.bass_utils import run_bass_kernel_spmd

B, S, D, H, HD = 4, 2048, 2048, 16, 128
HL = 8              # heads per core
ML = HL * HD        # local model dims (1024)
P = 128
NDT = D // P        # 16 contraction tiles
NST = S // P        # 16 seq tiles
NSC = S // 512      # 4 seq chunks
ISQ = 1.0 / math.sqrt(HD)
F32 = mybir.dt.float32
F32R = mybir.dt.float32r
Exp = mybir.ActivationFunctionType.Exp

_BUILT = {}


def _build():
    nc = bacc.Bacc("TRN2", target_bir_lowering=False, debug=False, num_devices=8)
    xT = nc.declare_dram_parameter("xT", [D, S], F32R, isOutput=False)
    wqT = nc.declare_dram_parameter("wqT", [D, ML], F32R, isOutput=False)
    wkT = nc.declare_dram_parameter("wkT", [D, ML], F32R, isOutput=False)
    wvT = nc.declare_dram_parameter("wvT", [D, ML], F32R, isOutput=False)
    woT = nc.declare_dram_parameter("woT", [ML, D], F32R, isOutput=False)
    mask0 = nc.declare_dram_parameter("mask0", [P, P], F32, isOutput=False)
    out = nc.declare_dram_parameter("out", [S, D], F32, isOutput=True)

    with tile.TileContext(nc) as tc, ExitStack() as top:
        dp = top.enter_context(tc.tile_pool(name="dram", bufs=1, space="DRAM"))
        # fine-grained scratch: one DRAM tile per (head, chunk) / (mq, half)
        sq = [[dp.tile([P, 512], F32R, tag=f"sq{h}_{c}", name=f"sq{h}_{c}")
               for c in range(NSC)] for h in range(HL)]
        sk = [[dp.tile([P, 512], F32R, tag=f"sk{h}_{c}", name=f"sk{h}_{c}")
               for c in range(NSC)] for h in range(HL)]
        sv = [[dp.tile([1024, 256], F32R, tag=f"sv{m}_{g}", name=f"sv{m}_{g}")
               for g in range(2)] for m in range(4)]
        sctx = [dp.tile([P, S], F32R, tag=f"sctx{h}", name=f"sctx{h}")
                for h in range(HL)]

        xT_r = xT.ap().rearrange("(t p) s -> p t s", p=P)
        wqT_r = wqT.ap().rearrange("(t p) m -> p t m", p=P)
        wkT_r = wkT.ap().rearrange("(t p) m -> p t m", p=P)
        wvT_r = wvT.ap().rearrange("(t p) m -> p t m", p=P)

        # phase-1 (left side) and phase-2 (right side) pools coexist
        p1 = ExitStack()
        xr_pool = p1.enter_context(
            tc.tile_pool(name="xr_pool", bufs=1, side="left"))
        vw = p1.enter_context(tc.tile_pool(name="vw", bufs=2, side="left"))
        qkw = p1.enter_context(tc.tile_pool(name="qkw", bufs=2, side="left"))
        p1ev = p1.enter_context(tc.tile_pool(name="p1ev", bufs=3, side="left"))
        p1ps = p1.enter_context(
            tc.tile_pool(name="p1ps", bufs=2, space="PSUM", side="left"))

        p2 = ExitStack()
        const = p2.enter_context(
            tc.tile_pool(name="const", bufs=1, side="right"))
        qk2 = p2.enter_context(tc.tile_pool(name="qk2", bufs=2, side="right"))
        p2w = p2.enter_context(tc.tile_pool(name="p2w", bufs=1, side="right"))
        p2ps = p2.enter_context(
            tc.tile_pool(name="p2ps", bufs=2, space="PSUM", side="right"))
        p2ps1 = p2.enter_context(
            tc.tile_pool(name="p2ps1", bufs=1, space="PSUM", side="right"))

        # attention constants, loaded up front
        m0 = const.tile([P, P], F32, tag="m0")
        nc.sync.dma_start(out=m0[:], in_=mask0[:])
        ones_f = const.tile([P, 1], F32, tag="ones_f")
        nc.vector.memset(ones_f[:], 1.0)
        ones_col = const.tile([P, 1], F32R, tag="ones_col")
        nc.vector.tensor_copy(ones_col[:], ones_f[:])
        onesr_f = const.tile([1, P], F32, tag="onesr_f")
        nc.vector.memset(onesr_f[:], 1.0)
        ones_row = const.tile([1, P], F32R, tag="ones_row")
        nc.vector.tensor_copy(ones_row[:], onesr_f[:])

        def attention_chunk(h, c, k_g, v_g, koff):
            """Emit attention for (head h, i-chunk c). k_g covers k chunks
            [0, koff) of the head; v_g covers the matching s-tiles."""
            nj = 4 * c + 4
            q_c = qk2.tile([P, 512], F32R, tag="q_c", bufs=3, name=f"q_{h}_{c}")
            nc.sync.dma_start(out=q_c[:], in_=sq[h][c][:])
            pctx = p2ps.tile([P, 512], F32, tag="pctx", name=f"pctx_{h}_{c}")
            pden = p2ps1.tile([1, 512], F32, tag="pden", name=f"pden_{h}_{c}")
            for J in range(nj):
                diag = J >= 4 * c
                r = J * P - c * 512 if diag else 0
                pscr = p2ps.tile([P, 512], F32, tag="pscr",
                                 name=f"pscr_{h}_{c}_{J}")
                nc.tensor.matmul(
                    pscr[:, r:512],
                    k_g[:, J * P:(J + 1) * P], q_c[:, r:512],
                    start=True, stop=True)
                pt = p2w.tile([P, 512], F32R, tag="pt", bufs=4,
                              name=f"pt_{h}_{c}_{J}")
                nc.scalar.activation(pt[:, r:512], pscr[:, r:512], Exp,
                                     scale=ISQ)
                if diag:
                    nc.vector.tensor_mul(
                        pt[:, r:r + P], pt[:, r:r + P], m0[:])
                nc.tensor.matmul(
                    pctx[:, r:512], v_g[:, J, :], pt[:, r:512],
                    start=(J == 0), stop=(J == nj - 1))
                nc.tensor.matmul(
                    pden[:, r:512], ones_col[:], pt[:, r:512],
                    start=(J == 0), stop=(J == nj - 1))
            # 1/denominator, broadcast to 128 partitions via K=1 matmul
            recip = p2w.tile([1, 512], F32, tag="recip", name=f"rc_{h}_{c}")
            nc.vector.reciprocal(recip[:], pden[:])
            recip_r = p2w.tile([1, 512], F32R, tag="recip_r",
                               name=f"rcr_{h}_{c}")
            nc.vector.tensor_copy(recip_r[:], recip[:])
            pbc = p2ps1.tile([P, 512], F32, tag="pbc", name=f"pbc_{h}_{c}")
            nc.tensor.matmul(pbc[:], ones_row[:], recip_r[:],
                             start=True, stop=True)
            rb = p2w.tile([P, 512], F32, tag="rb", name=f"rb_{h}_{c}")
            nc.vector.tensor_copy(rb[:], pbc[:])
            cv = p2w.tile([P, 512], F32R, tag="cv", bufs=2, name=f"cv_{h}_{c}")
            nc.vector.tensor_mul(cv[:], pctx[:], rb[:])
            nc.sync.dma_start(
                out=sctx[h][:, c * 512:(c + 1) * 512], in_=cv[:])

        for g in range(2):
            # ---- Phase 1 half g: x^T half resident [128, 16, 1024] ----
            xr = xr_pool.tile([P, NDT, 1024], F32R, tag="xr", name=f"xr{g}")
            for lc in range(2):
                nc.sync.dma_start(
                    out=xr[:, :, lc * 512:(lc + 1) * 512],
                    in_=xT_r[:, :, (2 * g + lc) * 512:(2 * g + lc + 1) * 512])

            # v: [s-tile 128, m-quarter 256]; stationary = x, moving = Wv
            for mq in range(4):
                vr = vw.tile([P, NDT, 256], F32R, tag="vr", name=f"vr{g}_{mq}")
                nc.sync.dma_start(
                    out=vr[:], in_=wvT_r[:, :, mq * 256:(mq + 1) * 256])
                for stl in range(8):
                    ps = p1ps.tile([P, 512], F32, tag="pp")
                    for d in range(NDT):
                        nc.tensor.matmul(
                            ps[:, 0:256], xr[:, d, stl * P:(stl + 1) * P],
                            vr[:, d, :],
                            start=(d == 0), stop=(d == NDT - 1))
                    ev = p1ev.tile([P, 512], F32R, tag="ev")
                    nc.scalar.copy(ev[:, 0:256], ps[:, 0:256])
                    nc.sync.dma_start(
                        out=sv[mq][g][stl * P:(stl + 1) * P, :],
                        in_=ev[:, 0:256])

            # q^T / k^T per head: [m-tile 128, s-chunk]
            for h in range(HL):
                for wsrc, dst in ((wqT_r, sq[h]), (wkT_r, sk[h])):
                    wr = qkw.tile([P, NDT, P], F32R, tag="wr")
                    nc.sync.dma_start(
                        out=wr[:], in_=wsrc[:, :, h * P:(h + 1) * P])
                    for lc in range(2):
                        ps = p1ps.tile([P, 512], F32, tag="pp")
                        for d in range(NDT):
                            nc.tensor.matmul(
                                ps[:], wr[:, d, :],
                                xr[:, d, lc * 512:(lc + 1) * 512],
                                start=(d == 0), stop=(d == NDT - 1))
                        ev = p1ev.tile([P, 512], F32R, tag="ev")
                        nc.scalar.copy(ev[:], ps[:])
                        nc.sync.dma_start(out=dst[2 * g + lc][:], in_=ev[:])

            # ---- Attention pass g: chunks 2g, 2g+1 for every head ----
            koff = 2 * (g + 1)  # k/v chunks needed: [0, koff)
            for h in range(HL):
                k_g = qk2.tile([P, koff * 512], F32R, tag=f"k{g}",
                               name=f"k{g}_{h}")
                for c in range(koff):
                    nc.sync.dma_start(
                        out=k_g[:, c * 512:(c + 1) * 512], in_=sk[h][c][:])
                v_g = qk2.tile([P, koff * 4, P], F32R, tag=f"v{g}",
                               name=f"v{g}_{h}")
                for gg in range(g + 1):
                    nc.sync.dma_start(
                        out=v_g[:, gg * 8:(gg + 1) * 8, :],
                        in_=sv[h // 2][gg][:].rearrange(
                            "(t p) m -> p t m", p=P)[
                            :, :, (h % 2) * P:(h % 2) * P + P])
                for lc in range(2):
                    attention_chunk(h, 2 * g + lc, k_g, v_g, koff)

        # free phase-1 space (x^T etc.); wo prefetch reuses it
        p1.close()

        # ---------------- Phase 3: out-projection ----------------
        with tc.tile_pool(name="wo_pool", bufs=1, side="left") as wo_pool, \
             tc.tile_pool(name="p3sb", bufs=3, side="left") as p3sb, \
             tc.tile_pool(name="p3ev", bufs=3, side="left") as p3ev, \
             tc.tile_pool(name="p3ps", bufs=2, space="PSUM",
                          side="left") as p3ps:
            woT_r = woT.ap().rearrange("(h p) n -> p h n", p=P)
            wo = wo_pool.tile([P, HL, D], F32R, tag="wo")
            for h in range(HL):
                nc.sync.dma_start(out=wo[:, h, :], in_=woT_r[:, h, :])
            for st in range(NST):
                cx = p3sb.tile([P, HL, P], F32R, tag="cx")
                for h in range(HL):
                    nc.sync.dma_start(
                        out=cx[:, h, :],
                        in_=sctx[h][:, st * P:(st + 1) * P])
                for nk in range(NSC):
                    ps = p3ps.tile([P, 512], F32, tag="po")
                    for h in range(HL):
                        nc.tensor.matmul(
                            ps[:], cx[:, h, :],
                            wo[:, h, nk * 512:(nk + 1) * 512],
                            start=(h == 0), stop=(h == HL - 1))
                    ev = p3ev.tile([P, 512], F32, tag="evo")
                    nc.scalar.copy(ev[:], ps[:])
                    nc.sync.dma_start(
                        out=out[st * P:(st + 1) * P,
                                nk * 512:(nk + 1) * 512], in_=ev[:])

        p2.close()

    nc.finalize()
    return nc


def get_nc():
    if "nc" not in _BUILT:
        _BUILT["nc"] = _build()
    return _BUILT["nc"]


def _to_f32r(a):
    """Round fp32 to the float32r (E8M11) grid: RNE at 12 low mantissa bits."""
    u = np.ascontiguousarray(a, dtype=np.float32).view(np.uint32)
    r = (u + 0x7FF + ((u >> 12) & 1)) & np.uint32(0xFFFFF000)
    return r.view(np.float32)


def _make_in_maps(x, Wq, Wk, Wv, Wo):
    jj, ff = np.meshgrid(np.arange(P), np.arange(P), indexing="ij")
    mask0 = (ff >= jj).astype(np.float32)
    in_maps = []
    for c in range(8):
        b, t = c // 2, c % 2
        ms = slice(t * ML, (t + 1) * ML)
        in_maps.append({
            "xT": _to_f32r(x[b].T),
            "wqT": _to_f32r(Wq[ms, :].T),
            "wkT": _to_f32r(Wk[ms, :].T),
            "wvT": _to_f32r(Wv[ms, :].T),
            "woT": _to_f32r(Wo[:, ms].T),
            "mask0": mask0,
        })
    return in_maps


def kernel(x, Wq, Wk, Wv, Wo):
    x = np.asarray(x, dtype=np.float32)
    Wq = np.asarray(Wq, dtype=np.float32)
    Wk = np.asarray(Wk, dtype=np.float32)
    Wv = np.asarray(Wv, dtype=np.float32)
    Wo = np.asarray(Wo, dtype=np.float32)

    nc = get_nc()
    in_maps = _make_in_maps(x, Wq, Wk, Wv, Wo)
    res = run_bass_kernel_spmd(nc, in_maps, list(range(8)))
    outs = [res.results[c]["out"] for c in range(8)]
    full = np.stack([outs[2 * b] + outs[2 * b + 1] for b in range(B)])
    return full.astype(np.float32)


# revision 18
# speedup vs baseline: 2.2404x; 1.4621x over previous
"""Causal self-attention (B=4, S=2048, D=2048, H=16) on 8 TRN2 NeuronCores.

Sharding: core c -> batch b=c//2, tensor-parallel half t=c%2 (8 heads each).
Each core computes QKV projections for its 8 heads, causal attention, and a
partial out-projection; the host sums the two TP partials per batch.

All heavy matmuls run in float32r (E8M11, 4x faster than fp32 on the PE,
~1.5e-4 relative rounding), accumulating in fp32 PSUM. Inputs are pre-rounded
to the f32r grid on the host, so no on-chip dtype conversions are needed.

Projections run in two sequence halves (x^T half resident at a time); since
causal attention on chunk c consumes only projections of chunks <= c, the
attention pass over the first half is emitted (and scheduled) between the two
projection halves. Phases communicate via per-(head, chunk) DRAM scratch;
attention context is spilled to DRAM and re-read by the out-projection, whose
weights prefetch into the space vacated by x^T.
"""
import math
from contextlib import ExitStack

import numpy as np

import concourse.bass as bass
import concourse.bacc as bacc
import concourse.mybir as mybir
import concourse.tile as tile
from concourse.bass_utils import run_bass_kernel_spmd

B, S, D, H, HD = 4, 2048, 2048, 16, 128
HL = 8              # heads per core
ML = HL * HD        # local model dims (1024)
P = 128
NDT = D // P        # 16 contraction tiles
NST = S // P        # 16 seq tiles
NSC = S // 512      # 4 seq chunks
ISQ = 1.0 / math.sqrt(HD)
F32 = mybir.dt.float32
F32R = mybir.dt.float32r
Exp = mybir.ActivationFunctionType.Exp

_BUILT = {}


def _build():
    nc = bacc.Bacc("TRN2", target_bir_lowering=False, debug=False, num_devices=8)
    xT = nc.declare_dram_parameter("xT", [D, S], F32R, isOutput=False)
    wqT = nc.declare_dram_parameter("wqT", [D, ML], F32R, isOutput=False)
    wkT = nc.declare_dram_parameter("wkT", [D, ML], F32R, isOutput=False)
    wvT = nc.declare_dram_parameter("wvT", [D, ML], F32R, isOutput=False)
    woT = nc.declare_dram_parameter("woT", [ML, D], F32R, isOutput=False)
    mask0 = nc.declare_dram_parameter("mask0", [P, P], F32, isOutput=False)
    out = nc.declare_dram_parameter("out", [S, D], F32, isOutput=True)

    with tile.TileContext(nc) as tc, ExitStack() as top:
        dp = top.enter_context(tc.tile_pool(name="dram", bufs=1, space="DRAM"))
        # fine-grained scratch: one DRAM tile per (head, chunk) / (mq, half)
        sq = [[dp.tile([P, 512], F32R, tag=f"sq{h}_{c}", name=f"sq{h}_{c}")
               for c in range(NSC)] for h in range(HL)]
        sk = [[dp.tile([P, 512], F32R, tag=f"sk{h}_{c}", name=f"sk{h}_{c}")
               for c in range(NSC)] for h in range(HL)]
        sv = [[dp.tile([1024, 256], F32R, tag=f"sv{m}_{g}", name=f"sv{m}_{g}")
               for g in range(2)] for m in range(4)]
        sctx = [dp.tile([P, S], F32R, tag=f"sctx{h}", name=f"sctx{h}")
                for h in range(HL)]

        xT_r = xT.ap().rearrange("(t p) s -> p t s", p=P)
        wqT_r = wqT.ap().rearrange("(t p) m -> p t m", p=P)
        wkT_r = wkT.ap().rearrange("(t p) m -> p t m", p=P)
        wvT_r = wvT.ap().rearrange("(t p) m -> p t m", p=P)

        # phase-1 (left side) and phase-2 (right side) pools coexist
        p1 = ExitStack()
        xr_pool = p1.enter_context(
            tc.tile_pool(name="xr_pool", bufs=1, side="left"))
        vw = p1.enter_context(tc.tile_pool(name="vw", bufs=2, side="left"))
        qkw = p1.enter_context(tc.tile_pool(name="qkw", bufs=2, side="left"))
        p1ev = p1.enter_context(tc.tile_pool(name="p1ev", bufs=3, side="left"))
        p1ps = p1.enter_context(
            tc.tile_pool(name="p1ps", bufs=2, space="PSUM", side="left"))

        p2 = ExitStack()
        const = p2.enter_context(
            tc.tile_pool(name="const", bufs=1, side="right"))
        qk2 = p2.enter_context(tc.tile_pool(name="qk2", bufs=2, side="right"))
        p2w = p2.enter_context(tc.tile_pool(name="p2w", bufs=1, side="right"))
        p2ps = p2.enter_context(
            tc.tile_pool(name="p2ps", bufs=2, space="PSUM", side="right"))
        p2ps1 = p2.enter_context(
            tc.tile_pool(name="p2ps1", bufs=1, space="PSUM", side="right"))

        # attention constants, loaded up front
        m0 = const.tile([P, P], F32, tag="m0")
        nc.sync.dma_start(out=m0[:], in_=mask0[:])
        ones_f = const.tile([P, 1], F32, tag="ones_f")
        nc.vector.memset(ones_f[:], 1.0)
        ones_col = const.tile([P, 1], F32R, tag="ones_col")
        nc.vector.tensor_copy(ones_col[:], ones_f[:])
        onesr_f = const.tile([1, P], F32, tag="onesr_f")
        nc.vector.memset(onesr_f[:], 1.0)
        ones_row = const.tile([1, P], F32R, tag="ones_row")
        nc.vector.tensor_copy(ones_row[:], onesr_f[:])

        def attention_chunk(h, c, k_g, v_g, koff):
            """Emit attention for (head h, i-chunk c). k_g covers k chunks
            [0, koff) of the head; v_g covers the matching s-tiles."""
            nj = 4 * c + 4
            ndiag = 4 * c  # J >= ndiag are diagonal tiles
            q_c = qk2.tile([P, 512], F32R, tag="q_c", bufs=3, name=f"q_{h}_{c}")
            nc.sync.dma_start(out=q_c[:], in_=sq[h][c][:])
            pctx = p2ps.tile([P, 512], F32, tag="pctx", name=f"pctx_{h}_{c}")
            pden = p2ps1.tile([1, 512], F32, tag="pden", name=f"pden_{h}_{c}")
            # denominator reduction: non-diagonal pt tiles are pre-summed in
            # quads on the DVE so the PE streams them once per quad
            n_pden = (ndiag + 3) // 4 + (nj - ndiag)
            pden_idx = 0
            ptsum = None
            quad = 0
            for J in range(nj):
                diag = J >= ndiag
                r = J * P - c * 512 if diag else 0
                pscr = p2ps.tile([P, 512], F32, tag="pscr",
                                 name=f"pscr_{h}_{c}_{J}")
                nc.tensor.matmul(
                    pscr[:, r:512],
                    k_g[:, J * P:(J + 1) * P], q_c[:, r:512],
                    start=True, stop=True)
                pt = p2w.tile([P, 512], F32R, tag="pt", bufs=7,
                              name=f"pt_{h}_{c}_{J}")
                nc.scalar.activation(pt[:, r:512], pscr[:, r:512], Exp,
                                     scale=ISQ)
                if diag:
                    nc.vector.tensor_mul(
                        pt[:, r:r + P], pt[:, r:r + P], m0[:])
                nc.tensor.matmul(
                    pctx[:, r:512], v_g[:, J, :], pt[:, r:512],
                    start=(J == 0), stop=(J == nj - 1))
                if diag:
                    nc.tensor.matmul(
                        pden[:, r:512], ones_col[:], pt[:, r:512],
                        start=(pden_idx == 0), stop=(pden_idx == n_pden - 1))
                    pden_idx += 1
                else:
                    if quad == 0:
                        ptsum = pt
                    else:
                        ptsum2 = p2w.tile([P, 512], F32R, tag="ptsum",
                                          bufs=2, name=f"pts_{h}_{c}_{J}")
                        nc.vector.tensor_add(ptsum2[:], ptsum[:], pt[:])
                        ptsum = ptsum2
                    quad += 1
                    if quad == 4 or J == ndiag - 1:
                        nc.tensor.matmul(
                            pden[:], ones_col[:], ptsum[:],
                            start=(pden_idx == 0),
                            stop=(pden_idx == n_pden - 1))
                        pden_idx += 1
                        quad = 0
                        ptsum = None
            # 1/denominator, broadcast to 128 partitions via K=1 matmul
            recip = p2w.tile([1, 512], F32, tag="recip", name=f"rc_{h}_{c}")
            nc.vector.reciprocal(recip[:], pden[:])
            recip_r = p2w.tile([1, 512], F32R, tag="recip_r",
                               name=f"rcr_{h}_{c}")
            nc.vector.tensor_copy(recip_r[:], recip[:])
            pbc = p2ps1.tile([P, 512], F32, tag="pbc", name=f"pbc_{h}_{c}")
            nc.tensor.matmul(pbc[:], ones_row[:], recip_r[:],
                             start=True, stop=True)
            rb = p2w.tile([P, 512], F32, tag="rb", name=f"rb_{h}_{c}")
            nc.vector.tensor_copy(rb[:], pbc[:])
            cv = p2w.tile([P, 512], F32R, tag="cv", bufs=2, name=f"cv_{h}_{c}")
            nc.vector.tensor_mul(cv[:], pctx[:], rb[:])
            nc.sync.dma_start(
                out=sctx[h][:, c * 512:(c + 1) * 512], in_=cv[:])

        for g in range(2):
            # ---- Phase 1 half g: x^T half resident [128, 16, 1024] ----
            xr = xr_pool.tile([P, NDT, 1024], F32R, tag="xr", name=f"xr{g}")
            if g == 0:
                # fine-grained first loads so the first v matmul group (which
                # needs only s-tile 0 and the first Wv quarter) starts early
                for q4 in range(4):
                    nc.sync.dma_start(
                        out=xr[:, :, q4 * P:(q4 + 1) * P],
                        in_=xT_r[:, :, q4 * P:(q4 + 1) * P])
                nc.sync.dma_start(out=xr[:, :, 512:1024],
                                  in_=xT_r[:, :, 512:1024])
            else:
                for lc in range(2):
                    nc.sync.dma_start(
                        out=xr[:, :, lc * 512:(lc + 1) * 512],
                        in_=xT_r[:, :, (2 * g + lc) * 512:
                                 (2 * g + lc + 1) * 512])

            # v: [s-tile 128, m-quarter 256]; stationary = x, moving = Wv
            for mq in range(4):
                vr = vw.tile([P, NDT, 256], F32R, tag="vr", name=f"vr{g}_{mq}")
                nc.sync.dma_start(
                    out=vr[:], in_=wvT_r[:, :, mq * 256:(mq + 1) * 256])
                for stl in range(8):
                    ps = p1ps.tile([P, 512], F32, tag="pp")
                    for d in range(NDT):
                        nc.tensor.matmul(
                            ps[:, 0:256], xr[:, d, stl * P:(stl + 1) * P],
                            vr[:, d, :],
                            start=(d == 0), stop=(d == NDT - 1))
                    ev = p1ev.tile([P, 512], F32R, tag="ev")
                    nc.scalar.copy(ev[:, 0:256], ps[:, 0:256])
                    nc.sync.dma_start(
                        out=sv[mq][g][stl * P:(stl + 1) * P, :],
                        in_=ev[:, 0:256])

            # q^T / k^T per head: [m-tile 128, s-chunk]
            for h in range(HL):
                for wsrc, dst in ((wqT_r, sq[h]), (wkT_r, sk[h])):
                    wr = qkw.tile([P, NDT, P], F32R, tag="wr")
                    nc.sync.dma_start(
                        out=wr[:], in_=wsrc[:, :, h * P:(h + 1) * P])
                    for lc in range(2):
                        ps = p1ps.tile([P, 512], F32, tag="pp")
                        for d in range(NDT):
                            nc.tensor.matmul(
                                ps[:], wr[:, d, :],
                                xr[:, d, lc * 512:(lc + 1) * 512],
                                start=(d == 0), stop=(d == NDT - 1))
                        ev = p1ev.tile([P, 512], F32R, tag="ev")
                        nc.scalar.copy(ev[:], ps[:])
                        nc.sync.dma_start(out=dst[2 * g + lc][:], in_=ev[:])

            # ---- Attention pass g: chunks 2g, 2g+1 for every head ----
            koff = 2 * (g + 1)  # k/v chunks needed: [0, koff)
            for h in range(HL):
                k_g = qk2.tile([P, koff * 512], F32R, tag=f"k{g}",
                               name=f"k{g}_{h}")
                for c in range(koff):
                    nc.sync.dma_start(
                        out=k_g[:, c * 512:(c + 1) * 512], in_=sk[h][c][:])
                v_g = qk2.tile([P, koff * 4, P], F32R, tag=f"v{g}",
                               name=f"v{g}_{h}")
                for gg in range(g + 1):
                    nc.sync.dma_start(
                        out=v_g[:, gg * 8:(gg + 1) * 8, :],
                        in_=sv[h // 2][gg][:].rearrange(
                            "(t p) m -> p t m", p=P)[
                            :, :, (h % 2) * P:(h % 2) * P + P])
                for lc in range(2):
                    attention_chunk(h, 2 * g + lc, k_g, v_g, koff)

        # free phase-1 space (x^T etc.); wo prefetch reuses it
        p1.close()

        # ---------------- Phase 3: out-projection ----------------
        with tc.tile_pool(name="wo_pool", bufs=1, side="left") as wo_pool, \
             tc.tile_pool(name="p3sb", bufs=3, side="left") as p3sb, \
             tc.tile_pool(name="p3ev", bufs=3, side="left") as p3ev, \
             tc.tile_pool(name="p3ps", bufs=2, space="PSUM",
                          side="left") as p3ps:
            woT_r = woT.ap().rearrange("(h p) n -> p h n", p=P)
            wo = wo_pool.tile([P, HL, D], F32R, tag="wo")
            for h in range(HL):
                nc.sync.dma_start(out=wo[:, h, :], in_=woT_r[:, h, :])
            for st in range(NST):
                cx = p3sb.tile([P, HL, P], F32R, tag="cx")
                for h in range(HL):
                    nc.sync.dma_start(
                        out=cx[:, h, :],
                        in_=sctx[h][:, st * P:(st + 1) * P])
                for nk in range(NSC):
                    ps = p3ps.tile([P, 512], F32, tag="po")
                    for h in range(HL):
                        nc.tensor.matmul(
                            ps[:], cx[:, h, :],
                            wo[:, h, nk * 512:(nk + 1) * 512],
                            start=(h == 0), stop=(h == HL - 1))
                    ev = p3ev.tile([P, 512], F32, tag="evo")
                    nc.scalar.copy(ev[:], ps[:])
                    nc.sync.dma_start(
                        out=out[st * P:(st + 1) * P,
                                nk * 512:(nk + 1) * 512], in_=ev[:])

        p2.close()

    nc.finalize()
    return nc


def get_nc():
    if "nc" not in _BUILT:
        _BUILT["nc"] = _build()
    return _BUILT["nc"]


def _to_f32r(a):
    """Round fp32 to the float32r (E8M11) grid: RNE at 12 low mantissa bits."""
    u = np.ascontiguousarray(a, dtype=np.float32).view(np.uint32)
    r = (u + 0x7FF + ((u >> 12) & 1)) & np.uint32(0xFFFFF000)
    return r.view(np.float32)


def _make_in_maps(x, Wq, Wk, Wv, Wo):
    jj, ff = np.meshgrid(np.arange(P), np.arange(P), indexing="ij")
    mask0 = (ff >= jj).astype(np.float32)
    in_maps = []
    for c in range(8):
        b, t = c // 2, c % 2
        ms = slice(t * ML, (t + 1) * ML)
        in_maps.append({
            "xT": _to_f32r(x[b].T),
            "wqT": _to_f32r(Wq[ms, :].T),
            "wkT": _to_f32r(Wk[ms, :].T),
            "wvT": _to_f32r(Wv[ms, :].T),
            "woT": _to_f32r(Wo[:, ms].T),
            "mask0": mask0,
        })
    return in_maps


def kernel(x, Wq, Wk, Wv, Wo):
    x = np.asarray(x, dtype=np.float32)
    Wq = np.asarray(Wq, dtype=np.float32)
    Wk = np.asarray(Wk, dtype=np.float32)
    Wv = np.asarray(Wv, dtype=np.float32)
    Wo = np.asarray(Wo, dtype=np.float32)

    nc = get_nc()
    in_maps = _make_in_maps(x, Wq, Wk, Wv, Wo)
    res = run_bass_kernel_spmd(nc, in_maps, list(range(8)))
    outs = [res.results[c]["out"] for c in range(8)]
    full = np.stack([outs[2 * b] + outs[2 * b + 1] for b in range(B)])
    return full.astype(np.float32)
